# revision 10
# baseline (speedup 1.0000x reference)
"""MinRNN Trainium2 kernel — parallel-in-time Jacobi iteration.

Model (per batch row b):
    z_t = tanh(x_t @ W_in^T + b_in)                      # no recurrence
    u_t = sigmoid(s_{t-1} @ W_rec^T + z_t @ U_z^T + b_u)
    s_t = u_t * s_{t-1} + (1 - u_t) * z_t

Reformulate with m_t := s_t - z_t  (so s_t = z_t + m_t):
    pre_t = atil_t + W_rec m_{t-1},   atil_t = W_rec z_{t-1} + U_z z_t + b_u
    m_t   = sigmoid(pre_t) * (zd_t + m_{t-1}),  zd_t = z_{t-1} - z_t
(z_{-1} = 0, m_{-1} = 0 gives s_{-1} = 0.)

The sequential scan is LDWEIGHTS-bound on the PE (~850ns/step streaming all
of W_rec against a 2-wide operand => ~1.75ms floor).  Instead solve the
whole-trajectory fixed point by Jacobi sweeps

    m^{(j)}_t = sigmoid(atil_t + W_rec m^{(j-1)}_{t-1}) * (zd_t + m^{(j-1)}_{t-1})

where each sweep is a dense GEMM over all T steps at once (512-wide tiles,
full PE efficiency, ~28us/sweep/core).  On this problem's data the error
contracts ~0.7x per sweep; 20 sweeps reach ~2e-3 max-rel (gate is 2e-2).
All sweep tensors are fp16 (bf16 plateaus at ~2e-2; fp16 at ~2e-3).

Sharding: data-parallel over batch, 2 rows per core, no collectives.
Everything except x / z16 / output lives in SBUF for the whole kernel.
"""

import numpy as np

import concourse.bass as bass
import concourse.mybir as mybir
import concourse.tile as tile
import concourse.bacc as bacc
from concourse import bass_utils

AF = mybir.ActivationFunctionType
ET = mybir.EngineType
ALU = mybir.AluOpType if hasattr(mybir, "AluOpType") else None

B, T, I, H = 16, 2048, 512, 512
N_CORES = 8
BL = B // N_CORES          # batch rows per core (2)
KC = I // 128              # input-dim chunks (4)
HC = H // 128              # hidden-dim chunks (4)
SWEEPS = 20                # total sweeps incl. the GEMM-free first one

f32 = mybir.dt.float32
f32r = mybir.dt.float32r
f16 = mybir.dt.float16


def build(t_steps: int = T, tb: int = 0, sweeps: int = SWEEPS, compile: bool = True,
          dbg: bool = False):
    """Build the per-core Bass program (same program on all 8 cores).
    `tb` accepted for CLI compat; tile width is fixed at min(512, T*BL)."""
    tw = min(512, t_steps * BL)
    assert (t_steps * BL) % tw == 0

    nc = bacc.Bacc("TRN2", target_bir_lowering=False, debug=False)

    xT = nc.dram_tensor("xT", [KC, 128, t_steps, BL], f32r, kind="ExternalInput")
    winT = nc.dram_tensor("winT", [KC, 128, H], f32r, kind="ExternalInput")
    wrecT = nc.dram_tensor("wrecT", [HC, 128, H], f16, kind="ExternalInput")
    uzT = nc.dram_tensor("uzT", [HC, 128, H], f16, kind="ExternalInput")
    bin2 = nc.dram_tensor("bin2", [HC, 128], f32, kind="ExternalInput")
    bu2 = nc.dram_tensor("bu2", [HC, 128], f32, kind="ExternalInput")
    out = nc.dram_tensor("outT", [HC, 128, t_steps, BL], f16, kind="ExternalOutput")
    dbg_t = None
    if dbg:
        dbg_t = {
            name: nc.dram_tensor(name, [HC, 128, t_steps, BL], f16,
                                 kind="ExternalOutput")
            for name in ("d_atil", "d_zd", "d_m1", "d_m2")
        }

    with tile.TileContext(nc) as tc:
        _body(tc, nc, xT, winT, wrecT, uzT, bin2, bu2, out, t_steps, tw, sweeps,
              dbg_t)

    if compile:
        nc.compile()
    return nc


def _body(tc, nc, xT, winT, wrecT, uzT, bin2, bu2, out, t_steps, tw, sweeps,
          dbg_t=None):
    from contextlib import ExitStack

    nt = (t_steps * BL) // tw          # number of t-tiles
    tws = tw // BL                     # steps per tile
    tp1 = t_steps + 1                  # padded length of m in t

    with ExitStack() as ctx:
        cpool = ctx.enter_context(tc.tile_pool(name="consts", bufs=1))
        mpool = ctx.enter_context(tc.tile_pool(name="master", bufs=1))
        xpool = ctx.enter_context(tc.tile_pool(name="xin", bufs=2))
        zpool = ctx.enter_context(tc.tile_pool(name="ztile", bufs=2))
        pqpool = ctx.enter_context(tc.tile_pool(name="psum", bufs=2, space="PSUM"))
        swpool = ctx.enter_context(tc.tile_pool(name="sweep", bufs=2))
        drpool = ctx.enter_context(tc.tile_pool(name="scratch", bufs=1, space="DRAM"))

        # ---- constants in SBUF ----
        w_in = cpool.tile([128, KC * H], f32r, tag="w_in")
        w_rec = cpool.tile([128, HC * H], f16, tag="w_rec")
        u_z = cpool.tile([128, HC * H], f16, tag="u_z")
        for k in range(KC):
            nc.sync.dma_start(w_in[:, k * H:(k + 1) * H], winT[k])
            nc.sync.dma_start(w_rec[:, k * H:(k + 1) * H], wrecT[k])
            nc.sync.dma_start(u_z[:, k * H:(k + 1) * H], uzT[k])
        binS = cpool.tile([128, HC], f32, tag="binS")
        nc.sync.dma_start(binS[:], bin2.ap().rearrange("c p -> p c"))
        buS = cpool.tile([128, HC], f32, tag="buS")
        nc.sync.dma_start(buS[:], bu2.ap().rearrange("c p -> p c"))

        # ---- SBUF masters ----
        # atil/zd: [128, (c, t, b)];  m ping/pong: [128, (c, 1+t, b)] (t=-1 pad)
        atil = mpool.tile([128, HC * t_steps * BL], f16, tag="atil")
        zd = mpool.tile([128, HC * t_steps * BL], f16, tag="zd")
        m_a = mpool.tile([128, HC * tp1 * BL], f16, tag="m_a")
        m_b = mpool.tile([128, HC * tp1 * BL], f16, tag="m_b")
        a3 = atil[:].rearrange("p (c t b) -> p c t b", c=HC, b=BL)
        zd3 = zd[:].rearrange("p (c t b) -> p c t b", c=HC, b=BL)
        ma3 = m_a[:].rearrange("p (c t b) -> p c t b", c=HC, b=BL)
        mb3 = m_b[:].rearrange("p (c t b) -> p c t b", c=HC, b=BL)
        # zero the t=-1 pad columns
        nc.vector.memset(ma3[:, :, 0, :], 0.0)
        nc.vector.memset(mb3[:, :, 0, :], 0.0)

        # z16 round-trips through DRAM for the final s = z + m pass
        zt_d = drpool.tile([HC, 128, t_steps, BL], f16, tag="zt_d")
        zt_ap = zt_d[:, :, :, :]

        xr = xT.ap().rearrange("k p t b -> p k t b")

        # carry: last t-column of the previous tile's z16 (per k-chunk)
        z_last = cpool.tile([128, KC * BL], f16, tag="z_last")
        nc.vector.memset(z_last[:], 0.0)
        zl2 = z_last[:].rearrange("p (k b) -> p k b", b=BL)

        # ================= phase A: z, atil, zd =================
        for ti in range(nt):
            t0 = ti * tws
            xs = xpool.tile([128, KC * tw], f32r, tag="xs")
            nc.sync.dma_start(
                xs[:].rearrange("p (k f) -> p k f", k=KC),
                xr[:, :, t0:t0 + tws, :],
            )
            # GEMM1: z = tanh(W_in x + b_in)
            psA = pqpool.tile([128, HC, tw], f32, tag="ps")
            for cm in range(HC):
                for k in range(KC):
                    nc.tensor.matmul(
                        psA[:, cm, :],
                        w_in[:, k * H + cm * 128:k * H + cm * 128 + 128],
                        xs[:, k * tw:(k + 1) * tw],
                        start=(k == 0),
                        stop=(k == KC - 1),
                        skip_group_check=True,
                    )
            z16 = zpool.tile([128, HC * tw], f16, tag="z16")
            z3 = z16[:].rearrange("p (c t b) -> p c t b", c=HC, b=BL)
            for cm in range(HC):
                nc.scalar.activation(z16[:, cm * tw:(cm + 1) * tw],
                                     psA[:, cm, :], AF.Tanh,
                                     bias=binS[:, cm:cm + 1], scale=1.0)
            nc.sync.dma_start(zt_ap[:, :, t0:t0 + tws, :]
                              .rearrange("c p t b -> p c t b"), z3)
            # zd = z_{t-1} - z_t   (boundary from carry)
            nc.vector.tensor_sub(zd3[:, :, t0, :], zl2[:, :, :], z3[:, :, 0, :])
            if tws > 1:
                nc.vector.tensor_sub(zd3[:, :, t0 + 1:t0 + tws, :],
                                     z3[:, :, 0:tws - 1, :],
                                     z3[:, :, 1:tws, :])
            # GEMM2: atil = U_z z_t + W_rec z_{t-1} + b_u
            psB = pqpool.tile([128, HC, tw], f32, tag="ps")
            for cm in range(HC):
                for k in range(HC):
                    nc.tensor.matmul(
                        psB[:, cm, :],
                        u_z[:, k * H + cm * 128:k * H + cm * 128 + 128],
                        z16[:, k * tw:(k + 1) * tw],
                        start=(k == 0),
                        stop=False,
                        skip_group_check=True,
                    )
                for k in range(HC):
                    wslice = w_rec[:, k * H + cm * 128:k * H + cm * 128 + 128]
                    nc.tensor.matmul(
                        psB[:, cm, BL:tw], wslice,
                        z16[:, k * tw:(k + 1) * tw - BL],
                        start=False, stop=(k == HC - 1),
                        skip_group_check=True,
                    )
                    nc.tensor.matmul(
                        psB[:, cm, 0:BL], wslice,
                        z_last[:, k * BL:(k + 1) * BL],
                        start=False, stop=(k == HC - 1),
                        skip_group_check=True,
                    )
                nc.scalar.activation(a3[:, cm, t0:t0 + tws, :].rearrange("p t b -> p (t b)"),
                                     psB[:, cm, :], AF.Identity,
                                     bias=buS[:, cm:cm + 1], scale=1.0)
            # update carry AFTER this tile's boundary matmuls consumed it
            nc.vector.tensor_copy(zl2[:, :, :], z3[:, :, tws - 1, :])

        # ================= sweep 1 (m=0 -> no GEMM) =================
        # m = sigmoid(atil) * zd
        for ti in range(nt):
            t0 = ti * tws
            u16 = swpool.tile([128, HC * tw], f16, tag="u16")
            nc.scalar.activation(
                u16[:].rearrange("p (c t b) -> p c t b", c=HC, b=BL),
                a3[:, :, t0:t0 + tws, :], AF.Sigmoid)
            nc.vector.tensor_mul(
                ma3[:, :, 1 + t0:1 + t0 + tws, :],
                u16[:].rearrange("p (c t b) -> p c t b", c=HC, b=BL),
                zd3[:, :, t0:t0 + tws, :])

        # ================= sweeps 2..sweeps =================
        def gemm_sweep(m_in3, m_out3, ti):
            """One t-tile of one Jacobi sweep: m_out = sig(atil + W m_in_shift)
            * (zd + m_in_shift).  m_in3/m_out3 are padded [p c (1+t) b] views."""
            t0 = ti * tws
            ps = pqpool.tile([128, HC, tw], f32, tag="ps")
            # W_rec @ m_{t-1}: shifted slice = padded cols [t0 .. t0+tws)
            for cm in range(HC):
                for k in range(HC):
                    nc.tensor.matmul(
                        ps[:, cm, :],
                        w_rec[:, k * H + cm * 128:k * H + cm * 128 + 128],
                        m_in3[:, k, t0:t0 + tws, :].rearrange("p t b -> p (t b)"),
                        start=(k == 0),
                        stop=(k == HC - 1),
                        skip_group_check=True,
                    )
            # pre = psum + atil  (in place in PSUM, f32)
            nc.vector.tensor_add(
                ps[:, :, :].rearrange("p c (t b) -> p c t b", b=BL),
                ps[:, :, :].rearrange("p c (t b) -> p c t b", b=BL),
                a3[:, :, t0:t0 + tws, :])
            u16 = swpool.tile([128, HC * tw], f16, tag="u16")
            nc.scalar.activation(u16[:], ps[:, :, :].rearrange("p c f -> p (c f)"),
                                 AF.Sigmoid)
            d16 = swpool.tile([128, HC * tw], f16, tag="d16")
            d3 = d16[:].rearrange("p (c t b) -> p c t b", c=HC, b=BL)
            nc.vector.tensor_add(d3[:, :, :, :],
                                 zd3[:, :, t0:t0 + tws, :],
                                 m_in3[:, :, t0:t0 + tws, :])
            nc.vector.tensor_mul(
                m_out3[:, :, 1 + t0:1 + t0 + tws, :],
                u16[:].rearrange("p (c t b) -> p c t b", c=HC, b=BL),
                d3[:, :, :, :])

        if dbg_t is not None:
            # dump atil, zd, m after sweep 1; one GEMM sweep; m after sweep 2
            nc.sync.dma_start(dbg_t["d_atil"].ap()
                              .rearrange("c p t b -> p c t b"), a3)
            nc.sync.dma_start(dbg_t["d_zd"].ap()
                              .rearrange("c p t b -> p c t b"), zd3)
            nc.sync.dma_start(dbg_t["d_m1"].ap()
                              .rearrange("c p t b -> p c t b"), ma3[:, :, 1:, :])
            for ti in range(nt):
                gemm_sweep(ma3, mb3, ti)
            nc.sync.dma_start(dbg_t["d_m2"].ap()
                              .rearrange("c p t b -> p c t b"), mb3[:, :, 1:, :])
            nc.sync.dma_start(out.ap()
                              .rearrange("c p t b -> p c t b"), mb3[:, :, 1:, :])
            return

        n_gemm = sweeps - 1            # sweeps after the GEMM-free first one
        n_pairs = (n_gemm - 1) // 2    # paired a->b->a sweeps in the hw loop
        rem = n_gemm - 1 - 2 * n_pairs # 0 or 1 extra before the final sweep

        if n_pairs > 0:
            with tc.For_i(0, n_pairs, 1,
                          hint_engines=(ET.PE, ET.DVE, ET.Activation),
                          name="sweeps") as _:
                for ti in range(nt):
                    gemm_sweep(ma3, mb3, ti)
                for ti in range(nt):
                    gemm_sweep(mb3, ma3, ti)
        if rem:
            for ti in range(nt):
                gemm_sweep(ma3, mb3, ti)
            m_pen, m_fin = mb3, ma3
        else:
            m_pen, m_fin = ma3, mb3

        # final sweep, fused with the output pass: s = z + m
        for ti in range(nt):
            t0 = ti * tws
            gemm_sweep(m_pen, m_fin, ti)
            zf = zpool.tile([128, HC * tw], f16, tag="zf")
            zf3 = zf[:].rearrange("p (c t b) -> p c t b", c=HC, b=BL)
            nc.sync.dma_start(zf3, zt_ap[:, :, t0:t0 + tws, :]
                              .rearrange("c p t b -> p c t b"))
            s16 = zpool.tile([128, HC * tw], f16, tag="s16")
            s3 = s16[:].rearrange("p (c t b) -> p c t b", c=HC, b=BL)
            nc.vector.tensor_add(s3, zf3, m_fin[:, :, 1 + t0:1 + t0 + tws, :])
            nc.sync.dma_start(out.ap()[:, :, t0:t0 + tws, :]
                              .rearrange("c p t b -> p c t b"), s3)


_CACHED = {}


def _get_nc(t_steps=T, sweeps=SWEEPS):
    key = (t_steps, sweeps)
    if key not in _CACHED:
        _CACHED[key] = build(t_steps, sweeps=sweeps)
    return _CACHED[key]


def make_in_maps(inputs, W_in, b_in, W_rec, U_z, b_u, t_steps=T):
    x = np.asarray(inputs, dtype=np.float32)
    winT_np = np.ascontiguousarray(
        np.asarray(W_in, np.float32).T.reshape(KC, 128, H))
    wrecT_np = np.ascontiguousarray(
        np.asarray(W_rec, np.float32).T.reshape(HC, 128, H)).astype(np.float16)
    uzT_np = np.ascontiguousarray(
        np.asarray(U_z, np.float32).T.reshape(HC, 128, H)).astype(np.float16)
    bin_np = np.ascontiguousarray(np.asarray(b_in, np.float32).reshape(HC, 128))
    bu_np = np.ascontiguousarray(np.asarray(b_u, np.float32).reshape(HC, 128))

    in_maps = []
    for c in range(N_CORES):
        xc = x[c * BL:(c + 1) * BL, :t_steps, :]          # (BL, t, I)
        xTc = np.ascontiguousarray(xc.transpose(2, 1, 0)  # (I, t, BL)
                                   ).reshape(KC, 128, t_steps, BL)
        in_maps.append({
            "xT": xTc, "winT": winT_np, "wrecT": wrecT_np, "uzT": uzT_np,
            "bin2": bin_np, "bu2": bu_np,
        })
    return in_maps


def kernel(inputs, W_in, b_in, W_rec, U_z, b_u):
    nc = _get_nc()
    in_maps = make_in_maps(inputs, W_in, b_in, W_rec, U_z, b_u)
    res = bass_utils.run_bass_kernel_spmd(nc, in_maps, core_ids=list(range(N_CORES)))
    outs = [unpack_out(res.results[c]["outT"]) for c in range(N_CORES)]
    return np.ascontiguousarray(np.concatenate(outs, axis=0), dtype=np.float32)


def unpack_out(oT):
    # [HC, 128, t, BL] -> [BL, t, HC*128]
    hc, p, t, bl = oT.shape
    return oT.transpose(3, 2, 0, 1).reshape(bl, t, hc * p).astype(np.float32)
